# revision 10
# baseline (speedup 1.0000x reference)
"""Trainium2 Bass kernel for nn_Dilateformer3 (multi-dilation local window attention).

Sharding: data-parallel over batch B=8 across 8 NeuronCores (1 image/core).

Per-core layout: channel-major activations [c_partitions, pixels].
 - padded K/V pixel grids 64x64 (real 56x56 at offset (+4,+4)) so all 9
   window shifts are plain free-dim offsets and out-of-image keys read 0.
 - scores s_j = sum_c Q*K_shift: DVE/GpSimd elementwise product + PE matmul
   with a block-diagonal ones lhsT reducing 96 (2 heads x 48c) partitions to 18.
 - softmax: zero-padded keys give score exactly 0 so exp=1, matching the
   reference (unfold zero-pads, softmax spans all 9 taps). Denominators are
   batched per branch into one [14,3,448]-tile reciprocal.
 - P broadcast 18->96 partitions via stride-0-source DMAs on the (otherwise
   idle) DMA engines; AV = contiguous bf16 TT mults + a DVE/GpSimd add tree.
 - qkv + final proj: PE matmuls (bf16 in, fp32 PSUM accum); proj bias is
   folded into the matmul as a rank-1 ones-row update; ACT evacuates.
"""

import sys
import numpy as np

for p in ("/opt/trn_rl_repo",):
    if p not in sys.path:
        sys.path.insert(0, p)

import ml_dtypes
import concourse.bass as bass
import concourse.bacc as bacc
import concourse.tile as tile
from concourse import mybir
from concourse.alu_op_type import AluOpType
from concourse.bass_utils import run_bass_kernel_spmd

BF16 = mybir.dt.bfloat16
F32 = mybir.dt.float32
NPBF16 = np.dtype(ml_dtypes.bfloat16)

B, H, W, C = 8, 56, 56, 288
NPIX = H * W                 # 3136
DILS = (1, 2, 3)
HD = 48                      # head dim
NBLK = 7                     # pixel blocks per branch
BLK = 448                    # = 8 rows x 56 cols
PW = 64                      # padded width
PH = 64                      # padded height
NPAD = PW * PH               # 4096
PADR, PADC = 4, 4            # top/left pad
SCALE = HD ** -0.5
SHIFTS = [(a, b) for a in (-1, 0, 1) for b in (-1, 0, 1)]  # (kh-1, kw-1) order
N_CORES = 8

# tuning knobs
GP_PRODUCT_TAPS = (7, 8)     # QK product taps computed on GpSimd (rest on DVE)


def _bcast_dma(eng, dst, src_row, nrep, length):
    """DMA-replicate a single-partition row [1, length] to nrep partitions.

    HWDGE rejects stride-0 partition dims, so the replication is expressed as
    a stride-0 *free* dim on the 1-partition source; dst gets a dummy middle
    dim so both sides stream (rep, pix) in the same order.
    """
    src_b = bass.AP(tensor=src_row.tensor, offset=src_row.offset,
                    ap=[list(src_row.ap[0]), [0, nrep], [1, length]])
    dst3 = bass.AP(tensor=dst.tensor, offset=dst.offset,
                   ap=[list(dst.ap[0]), [length, 1], [1, length]])
    eng.dma_start(out=dst3, in_=src_b)


def build_nc():
    nc = bacc.Bacc("TRN2", target_bir_lowering=False)

    def din(name, shape, dtype):
        return nc.dram_tensor(name, shape, dtype, kind="ExternalInput")

    xT = din("xT", [96, 3, NPIX], BF16)          # [cin%96, cin_chunk, px]
    wq = din("wq", [96, 9, 3, 96], BF16)         # [cin, m_chunk, k_chunk, cout]
    bq = din("bq", [96, 9], F32)                 # [cout%96, m_chunk]
    wp = din("wp", [96, 3, C], BF16)             # [cin%96, branch, cout]
    bp = din("bp", [1, C], BF16)                 # b_proj single row
    ones18 = din("ones18", [96, 9, 18], BF16)    # per-tap head-sum reduction lhsT
    selden = din("selden", [18, 18], BF16)       # denominator replication lhsT
    ones1 = din("ones1", [1, 128], BF16)         # proj bias rank-1 lhsT
    y = nc.dram_tensor("y", [NPIX, C], F32, kind="ExternalOutput")

    from contextlib import ExitStack
    with tile.TileContext(nc) as tc, ExitStack() as ctx:
        singles = ctx.enter_context(tc.tile_pool(name="singles", bufs=1))
        prodp = ctx.enter_context(tc.tile_pool(name="prodp", bufs=5))
        smallp = ctx.enter_context(tc.tile_pool(name="smallp", bufs=4))
        tmpp = ctx.enter_context(tc.tile_pool(name="tmpp", bufs=2))
        addp = ctx.enter_context(tc.tile_pool(name="addp", bufs=4))
        rpp = ctx.enter_context(tc.tile_pool(name="rpp", bufs=2))
        yop = ctx.enter_context(tc.tile_pool(name="yop", bufs=2))
        bigp = ctx.enter_context(tc.tile_pool(name="bigp", bufs=2, space="PSUM"))
        ppb = ctx.enter_context(tc.tile_pool(name="ppb", bufs=3, space="PSUM"))

        # ---- persistent SBUF tensors ----
        xT_sb = singles.tile([96, 3, NPIX], BF16)
        wq_sb = singles.tile([96, 9, 3, 96], BF16)
        bq_sb = singles.tile([96, 9], F32)
        wp_sb = singles.tile([96, 3, C], BF16)
        bp_sb = singles.tile([1, C], BF16)
        ones18_sb = singles.tile([96, 9, 18], BF16)
        selden_sb = singles.tile([18, 18], BF16)
        ones1_sb = singles.tile([1, 128], BF16)
        Q_sb = singles.tile([96, 3, NPIX], BF16)
        K_sb = singles.tile([96, 3, NPAD], BF16)
        V_sb = singles.tile([96, 3, NPAD], BF16)
        Pc_sb = singles.tile([18, 3, NPIX], BF16)   # exp'd scores -> normalized probs
        den_sb = singles.tile([14, 3, BLK], F32)    # per-branch denominators
        rec_sb = singles.tile([14, 3, BLK], BF16)   # bf16 reciprocals
        yb_sb = singles.tile([96, 3, NPIX], BF16)   # attention out (c-major)

        # ---- load inputs ----
        nc.sync.dma_start(out=xT_sb, in_=xT[:, :, :])
        nc.sync.dma_start(out=wq_sb, in_=wq[:, :, :, :])
        nc.sync.dma_start(out=bq_sb, in_=bq[:, :])
        nc.sync.dma_start(out=wp_sb, in_=wp[:, :, :])
        nc.sync.dma_start(out=bp_sb, in_=bp[:, :])
        nc.sync.dma_start(out=ones18_sb, in_=ones18[:, :, :])
        nc.sync.dma_start(out=selden_sb, in_=selden[:, :])
        nc.sync.dma_start(out=ones1_sb, in_=ones1[:, :])

        # ---- zero K/V pad regions ----
        for t in (K_sb, V_sb):
            for br in range(3):
                g = t[:, br, :].rearrange("p (r c) -> p r c", c=PW)
                nc.vector.memset(g[:, 0:PADR, :], 0.0)                 # top rows
                nc.vector.memset(g[:, PADR + H:PH, :], 0.0)            # bottom rows
                nc.vector.memset(g[:, PADR:PADR + H, 0:PADC], 0.0)     # left pad
                nc.vector.memset(g[:, PADR:PADR + H, PADC + W:PW], 0.0)  # right pad

        # Wait-absorbers: walrus limits sem waits per instruction; these tiny
        # reads make each engine observe the setup DMA + memset ticks once.
        warm_a = smallp.tile([1, 8], F32, tag="warma")
        nc.scalar.activation(out=warm_a, in_=V_sb[0:1, 2, 1980:1988],
                             func=mybir.ActivationFunctionType.Copy)
        warm_a2 = smallp.tile([1, 8], F32, tag="warma")
        nc.scalar.activation(out=warm_a2, in_=bq_sb[0:1, 0:8],
                             func=mybir.ActivationFunctionType.Copy)
        warm_v = smallp.tile([1, 8], F32, tag="warma")
        nc.vector.tensor_copy(warm_v, bq_sb[0:1, 0:8])
        warm_v2 = smallp.tile([1, 8], F32, tag="warma")
        nc.vector.tensor_copy(warm_v2, V_sb[0:1, 2, 1980:1988])
        warm_g = smallp.tile([1, 8], F32, tag="warma")
        nc.gpsimd.tensor_copy(warm_g, bq_sb[0:1, 0:8])
        warm_g2 = smallp.tile([1, 8], F32, tag="warma")
        nc.gpsimd.tensor_copy(warm_g2, V_sb[0:1, 2, 1980:1988])

        # ---- phase 1: qkv projection ----
        for m in range(9):
            qkv_t, br = divmod(m, 3)  # 0=q 1=k 2=v
            for t in range(NBLK):
                ps_full = bigp.tile([128, BLK], F32, tag="big")
                ps = ps_full[0:96, :]
                for k in range(3):
                    nc.tensor.matmul(
                        ps,
                        lhsT=wq_sb[:, m, k, :],
                        rhs=xT_sb[:, k, t * BLK:(t + 1) * BLK],
                        start=(k == 0),
                        stop=(k == 2),
                    )
                if qkv_t == 0:
                    dest = Q_sb[:, br, t * BLK:(t + 1) * BLK]
                    src = ps
                else:
                    tgt = K_sb if qkv_t == 1 else V_sb
                    dest = tgt[:, br, :].rearrange("p (r c) -> p r c", c=PW)[
                        :, 8 * t + PADR:8 * t + 8 + PADR, PADC:PADC + W
                    ]
                    src = ps.rearrange("p (r c) -> p r c", c=W)
                if (m + t) % 2 == 0:
                    nc.scalar.activation(
                        out=dest, in_=src,
                        func=mybir.ActivationFunctionType.Identity,
                        bias=bq_sb[:, m:m + 1], scale=1.0,
                    )
                else:
                    nc.vector.tensor_scalar_add(dest, src, bq_sb[:, m:m + 1])

        # ---- phase 2A: scores + exp + denominators ----
        for br in range(3):
            d = DILS[br]
            Kg = K_sb[:, br, :].rearrange("p (r c) -> p r c", c=PW)
            for t in range(NBLK):
                sl = slice(t * BLK, (t + 1) * BLK)
                sp = ppb.tile([18, BLK], F32, tag="s18")
                q_in = Q_sb[:, br, sl].rearrange("p (r c) -> p r c", c=W)
                for j, (a, b) in enumerate(SHIFTS):
                    prod = prodp.tile([96, 8, W], BF16, tag="prod")
                    k_in = Kg[:, 8 * t + PADR + a * d: 8 * t + 8 + PADR + a * d,
                              PADC + b * d: PADC + b * d + W]
                    eng = nc.gpsimd if j in GP_PRODUCT_TAPS else nc.vector
                    eng.tensor_tensor(prod, q_in, k_in, op=AluOpType.mult)
                    nc.tensor.matmul(
                        sp,
                        lhsT=ones18_sb[:, j, :],
                        rhs=prod.rearrange("p r c -> p (r c)"),
                        start=(j == 0), stop=(j == 8),
                    )
                # exp (scale folded in)
                nc.scalar.activation(
                    out=Pc_sb[:, br, sl], in_=sp,
                    func=mybir.ActivationFunctionType.Exp, scale=SCALE,
                )
                # denominator, born replicated over the 18 rows
                dp = ppb.tile([18, BLK], F32, tag="s18")
                nc.tensor.matmul(dp, lhsT=selden_sb, rhs=Pc_sb[:, br, sl],
                                 start=True, stop=True)
                # pack head denominators into the per-branch tile; engines
                # can't write at partition base 2t, so stage + DMA.
                dst2 = smallp.tile([2, BLK], F32, tag="dst2")
                nc.scalar.activation(
                    out=dst2, in_=dp[0:2, :],
                    func=mybir.ActivationFunctionType.Copy,
                )
                nc.scalar.dma_start(out=den_sb[2 * t:2 * t + 2, br, :],
                                    in_=dst2)
            # batched reciprocal for the whole branch (14 rows x 448)
            nc.vector.reciprocal(den_sb[:, br, :], den_sb[:, br, :])
            nc.vector.tensor_copy(rec_sb[:, br, :], den_sb[:, br, :])

        # ---- phase 2B + 3: normalize, broadcast, AV ----
        for br in range(3):
            d = DILS[br]
            Vg = V_sb[:, br, :].rearrange("p (r c) -> p r c", c=PW)
            for t in range(NBLK):
                sl = slice(t * BLK, (t + 1) * BLK)
                # rec96: per-head reciprocal rows broadcast to 48 lanes each
                rec96 = rpp.tile([96, BLK], BF16, tag="rec96")
                for h in range(2):
                    _bcast_dma(nc.sync, rec96[48 * h:48 * h + 48, :],
                               rec_sb[2 * t + h:2 * t + h + 1, br, :], 48, BLK)
                # broadcast each (tap, head) exp'd-score row to its 48 lanes
                tmp = tmpp.tile([96, 9, BLK], BF16, tag="tmp")
                for j in range(9):
                    for h in range(2):
                        eng = nc.sync if (j + h) % 2 == 0 else nc.scalar
                        _bcast_dma(eng, tmp[48 * h:48 * h + 48, j, :],
                                   Pc_sb[2 * j + h:2 * j + h + 1, br, sl],
                                   48, BLK)
                # AV products, one op per kernel-row group (3 taps each)
                for a in range(3):
                    v_in = Vg[:, 8 * t + PADR + (a - 1) * d:
                              8 * t + 8 + PADR + (a - 1) * d,
                              PADC - d:PADC - d + W]
                    v_ap = bass.AP(
                        tensor=v_in.tensor, offset=v_in.offset,
                        ap=[list(v_in.ap[0]),
                            [d, 3], [PW, 8], [1, W]],
                    )
                    tslc = tmp[:, 3 * a:3 * a + 3, :].rearrange(
                        "p t (r c) -> p t r c", c=W)
                    nc.vector.tensor_tensor(tslc, tslc, v_ap, op=AluOpType.mult)
                # add tree: 9 taps -> 1, split DVE/GpSimd
                u_lo = addp.tile([96, 2, BLK], BF16, tag="add")
                nc.gpsimd.tensor_tensor(u_lo, tmp[:, 0:2, :], tmp[:, 2:4, :],
                                        op=AluOpType.add)
                u_hi = addp.tile([96, 2, BLK], BF16, tag="add")
                nc.vector.tensor_tensor(u_hi, tmp[:, 4:6, :], tmp[:, 6:8, :],
                                        op=AluOpType.add)
                v_t = addp.tile([96, 2, BLK], BF16, tag="add")
                nc.vector.tensor_tensor(v_t, u_lo, u_hi, op=AluOpType.add)
                w_t = addp.tile([96, 1, BLK], BF16, tag="addw")
                nc.vector.tensor_tensor(w_t, v_t[:, 0:1, :], v_t[:, 1:2, :],
                                        op=AluOpType.add)
                s_t = addp.tile([96, 1, BLK], BF16, tag="addw")
                with nc.allow_low_precision(reason="9-tap bf16 tree sum"):
                    nc.vector.tensor_tensor(
                        s_t, w_t, tmp[:, 8:9, :], op=AluOpType.add)
                # normalize by the softmax denominator at the very end
                nc.vector.tensor_tensor(yb_sb[:, br, sl], s_t[:, 0, :], rec96,
                                        op=AluOpType.mult)

        # ---- phase 4: output projection + write out ----
        for t in range(25):
            size = min(128, NPIX - t * 128)
            py_full = bigp.tile([128, BLK], F32, tag="big")
            py = py_full[:, 0:C]
            for br in range(3):
                nc.tensor.matmul(
                    py[:size, :],
                    lhsT=yb_sb[:, br, t * 128:t * 128 + size],
                    rhs=wp_sb[:, br, :],
                    start=(br == 0), stop=False,
                )
            # rank-1 bias add: ones column x b_proj row
            nc.tensor.matmul(
                py[:size, :],
                lhsT=ones1_sb[:, 0:size],
                rhs=bp_sb[:, :],
                start=False, stop=True,
            )
            yo_t = yop.tile([128, C], F32, tag="yo")
            nc.scalar.activation(out=yo_t[:size, :], in_=py[:size, :],
                                 func=mybir.ActivationFunctionType.Copy)
            nc.sync.dma_start(out=y[t * 128:t * 128 + size, :],
                              in_=yo_t[:size, :])

    nc.compile()
    return nc


def host_inputs(x, w_qkv, b_qkv, w_proj, b_proj):
    """Numpy prep of per-core + shared input arrays (keys match dram names)."""
    x = np.asarray(x, np.float32)
    w_qkv = np.asarray(w_qkv, np.float32)
    b_qkv = np.asarray(b_qkv, np.float32)
    w_proj = np.asarray(w_proj, np.float32)
    b_proj = np.asarray(b_proj, np.float32)

    # xT per core: [96, 3, NPIX]
    xT_all = x.reshape(B, NPIX, C).transpose(0, 2, 1)          # [B, C, NPIX]
    xT_all = xT_all.reshape(B, 3, 96, NPIX).transpose(0, 2, 1, 3)  # [B,96,3,NPIX]
    xT_all = np.ascontiguousarray(xT_all).astype(NPBF16)

    # wq: [cin96, m, k, cout96] = w_qkv[m*96+cout, k*96+cin]
    w3 = w_qkv.reshape(9, 96, 3, 96)                            # [m,cout,k,cin]
    wq_h = np.ascontiguousarray(w3.transpose(3, 0, 2, 1)).astype(NPBF16)
    bq_h = np.ascontiguousarray(b_qkv.reshape(9, 96).T).astype(np.float32)

    # wp: [cin96, branch, cout] = w_proj[cout, branch*96+cin]
    wp_h = np.ascontiguousarray(
        w_proj.reshape(C, 3, 96).transpose(2, 1, 0)).astype(NPBF16)
    bp_h = np.ascontiguousarray(b_proj[None, :]).astype(NPBF16)

    ones18_h = np.zeros((96, 9, 18), NPBF16)
    for j in range(9):
        ones18_h[0:48, j, 2 * j] = 1
        ones18_h[48:96, j, 2 * j + 1] = 1

    selden_h = np.zeros((18, 18), NPBF16)
    for j in range(9):
        for h in range(2):
            for j2 in range(9):
                selden_h[2 * j + h, 2 * j2 + h] = 1

    ones1_h = np.ones((1, 128), NPBF16)

    shared = dict(wq=wq_h, bq=bq_h, wp=wp_h, bp=bp_h, ones18=ones18_h,
                  selden=selden_h, ones1=ones1_h)
    in_maps = [dict(shared, xT=np.ascontiguousarray(xT_all[i]))
               for i in range(N_CORES)]
    return in_maps


_NC_CACHE = {}


def kernel(x, w_qkv, b_qkv, w_proj, b_proj):
    if "nc" not in _NC_CACHE:
        _NC_CACHE["nc"] = build_nc()
    nc = _NC_CACHE["nc"]
    in_maps = host_inputs(x, w_qkv, b_qkv, w_proj, b_proj)
    res = run_bass_kernel_spmd(nc, in_maps, list(range(N_CORES)))
    out = np.stack([res.results[i]["y"] for i in range(N_CORES)], axis=0)
    return out.reshape(B, H, W, C).astype(np.float32)


if __name__ == "__main__":
    print("built nc ok" if build_nc() else "")


# revision 13
# speedup vs baseline: 1.0425x; 1.0425x over previous
"""Trainium2 Bass kernel for nn_Dilateformer3 (multi-dilation local window attention).

Sharding: data-parallel over batch B=8 across 8 NeuronCores (1 image/core).

Per-core layout: channel-major activations [c_partitions, pixels].
 - padded K/V pixel grids 64x64 (real 56x56 at offset (+4,+4)) so all 9
   window shifts are plain free-dim offsets and out-of-image keys read 0.
 - scores s_j = sum_c Q*K_shift: DVE/GpSimd elementwise product + PE matmul
   with a block-diagonal ones lhsT reducing 96 (2 heads x 48c) partitions to 18.
 - softmax: zero-padded keys give score exactly 0 so exp=1, matching the
   reference (unfold zero-pads, softmax spans all 9 taps). Denominators are
   batched per branch into one [14,3,448]-tile reciprocal.
 - P broadcast 18->96 partitions via stride-0-source DMAs on the (otherwise
   idle) DMA engines; AV = contiguous bf16 TT mults + a DVE/GpSimd add tree.
 - qkv + final proj: PE matmuls (bf16 in, fp32 PSUM accum); proj bias is
   folded into the matmul as a rank-1 ones-row update; ACT evacuates.
"""

import sys
import numpy as np

for p in ("/opt/trn_rl_repo",):
    if p not in sys.path:
        sys.path.insert(0, p)

import ml_dtypes
import concourse.bass as bass
import concourse.bacc as bacc
import concourse.tile as tile
from concourse import mybir
from concourse.alu_op_type import AluOpType
from concourse.bass_utils import run_bass_kernel_spmd

BF16 = mybir.dt.bfloat16
F32 = mybir.dt.float32
NPBF16 = np.dtype(ml_dtypes.bfloat16)

B, H, W, C = 8, 56, 56, 288
NPIX = H * W                 # 3136
DILS = (1, 2, 3)
HD = 48                      # head dim
NBLK = 7                     # pixel blocks per branch
BLK = 448                    # = 8 rows x 56 cols
PW = 64                      # padded width
PH = 64                      # padded height
NPAD = PW * PH               # 4096
PADR, PADC = 4, 4            # top/left pad
SCALE = HD ** -0.5
SHIFTS = [(a, b) for a in (-1, 0, 1) for b in (-1, 0, 1)]  # (kh-1, kw-1) order
N_CORES = 8

def _bcast_dma(eng, dst, src_row, nrep, length):
    """DMA-replicate a single-partition row [1, length] to nrep partitions.

    HWDGE rejects stride-0 partition dims, so the replication is expressed as
    a stride-0 *free* dim on the 1-partition source; dst gets a dummy middle
    dim so both sides stream (rep, pix) in the same order.
    """
    src_b = bass.AP(tensor=src_row.tensor, offset=src_row.offset,
                    ap=[list(src_row.ap[0]), [0, nrep], [1, length]])
    dst3 = bass.AP(tensor=dst.tensor, offset=dst.offset,
                   ap=[list(dst.ap[0]), [length, 1], [1, length]])
    eng.dma_start(out=dst3, in_=src_b)


def build_nc():
    nc = bacc.Bacc("TRN2", target_bir_lowering=False)

    def din(name, shape, dtype):
        return nc.dram_tensor(name, shape, dtype, kind="ExternalInput")

    xT = din("xT", [96, 3, NPIX], BF16)          # [cin%96, cin_chunk, px]
    wq = din("wq", [96, 9, 3, 96], BF16)         # [cin, m_chunk, k_chunk, cout]
    bq = din("bq", [96, 9], F32)                 # [cout%96, m_chunk]
    wp = din("wp", [96, 3, C], BF16)             # [cin%96, branch, cout]
    bp = din("bp", [1, C], BF16)                 # b_proj single row
    ones18 = din("ones18", [96, 9, 18], BF16)    # per-tap head-sum reduction lhsT
    selden = din("selden", [18, 18], BF16)       # denominator replication lhsT
    ones1 = din("ones1", [1, 128], BF16)         # proj bias rank-1 lhsT
    y = nc.dram_tensor("y", [NPIX, C], F32, kind="ExternalOutput")

    from contextlib import ExitStack
    with tile.TileContext(nc) as tc, ExitStack() as ctx:
        singles = ctx.enter_context(tc.tile_pool(name="singles", bufs=1))
        prodp = ctx.enter_context(tc.tile_pool(name="prodp", bufs=5))
        smallp = ctx.enter_context(tc.tile_pool(name="smallp", bufs=4))
        dstp = ctx.enter_context(tc.tile_pool(name="dstp", bufs=3))
        pcp = ctx.enter_context(tc.tile_pool(name="pcp", bufs=2))
        denp = ctx.enter_context(tc.tile_pool(name="denp", bufs=2))
        recp = ctx.enter_context(tc.tile_pool(name="recp", bufs=2))
        tmpp = ctx.enter_context(tc.tile_pool(name="tmpp", bufs=2))
        rpp = ctx.enter_context(tc.tile_pool(name="rpp", bufs=2))
        yop = ctx.enter_context(tc.tile_pool(name="yop", bufs=2))
        bigp = ctx.enter_context(tc.tile_pool(name="bigp", bufs=2, space="PSUM"))
        ppb = ctx.enter_context(tc.tile_pool(name="ppb", bufs=4, space="PSUM"))

        # ---- persistent SBUF tensors ----
        xT_sb = singles.tile([96, 3, NPIX], BF16)
        wq_sb = singles.tile([96, 9, 3, 96], BF16)
        bq_sb = singles.tile([96, 9], F32)
        wp_sb = singles.tile([96, 3, C], BF16)
        bp_sb = singles.tile([1, C], BF16)
        ones18_sb = singles.tile([96, 9, 18], BF16)
        selden_sb = singles.tile([18, 18], BF16)
        ones1_sb = singles.tile([1, 128], BF16)
        Q_sb = singles.tile([96, 3, NPIX], BF16)
        K_sb = singles.tile([96, 3, NPAD], BF16)
        V_sb = singles.tile([96, 3, NPAD], BF16)
        yb_sb = singles.tile([96, 3, NPIX], BF16)   # attention out (c-major)

        # ---- load inputs ----
        nc.sync.dma_start(out=xT_sb, in_=xT[:, :, :])
        nc.sync.dma_start(out=wq_sb, in_=wq[:, :, :, :])
        nc.sync.dma_start(out=bq_sb, in_=bq[:, :])
        nc.sync.dma_start(out=wp_sb, in_=wp[:, :, :])
        nc.sync.dma_start(out=bp_sb, in_=bp[:, :])
        nc.sync.dma_start(out=ones18_sb, in_=ones18[:, :, :])
        nc.sync.dma_start(out=selden_sb, in_=selden[:, :])
        nc.sync.dma_start(out=ones1_sb, in_=ones1[:, :])

        # ---- zero K/V pad regions ----
        for t in (K_sb, V_sb):
            for br in range(3):
                g = t[:, br, :].rearrange("p (r c) -> p r c", c=PW)
                nc.vector.memset(g[:, 0:PADR, :], 0.0)                 # top rows
                nc.vector.memset(g[:, PADR + H:PH, :], 0.0)            # bottom rows
                nc.vector.memset(g[:, PADR:PADR + H, 0:PADC], 0.0)     # left pad
                nc.vector.memset(g[:, PADR:PADR + H, PADC + W:PW], 0.0)  # right pad

        # Wait-absorbers: walrus limits sem waits per instruction; these tiny
        # reads make each engine observe the setup DMA + memset ticks once.
        warm_a = smallp.tile([1, 8], F32, tag="warma")
        nc.scalar.activation(out=warm_a, in_=V_sb[0:1, 2, 1980:1988],
                             func=mybir.ActivationFunctionType.Copy)
        warm_a2 = smallp.tile([1, 8], F32, tag="warma")
        nc.scalar.activation(out=warm_a2, in_=bq_sb[0:1, 0:8],
                             func=mybir.ActivationFunctionType.Copy)
        warm_v = smallp.tile([1, 8], F32, tag="warma")
        nc.vector.tensor_copy(warm_v, bq_sb[0:1, 0:8])
        warm_v2 = smallp.tile([1, 8], F32, tag="warma")
        nc.vector.tensor_copy(warm_v2, V_sb[0:1, 2, 1980:1988])
        warm_g = smallp.tile([1, 8], F32, tag="warma")
        nc.gpsimd.tensor_copy(warm_g, bq_sb[0:1, 0:8])
        warm_g2 = smallp.tile([1, 8], F32, tag="warma")
        nc.gpsimd.tensor_copy(warm_g2, V_sb[0:1, 2, 1980:1988])

        # ---- phase 1: qkv projection ----
        for m in range(9):
            qkv_t, br = divmod(m, 3)  # 0=q 1=k 2=v
            for t in range(NBLK):
                ps_full = bigp.tile([128, BLK], F32, tag="big")
                ps = ps_full[0:96, :]
                for k in range(3):
                    nc.tensor.matmul(
                        ps,
                        lhsT=wq_sb[:, m, k, :],
                        rhs=xT_sb[:, k, t * BLK:(t + 1) * BLK],
                        start=(k == 0),
                        stop=(k == 2),
                    )
                if qkv_t == 0:
                    dest = Q_sb[:, br, t * BLK:(t + 1) * BLK]
                    src = ps
                else:
                    tgt = K_sb if qkv_t == 1 else V_sb
                    dest = tgt[:, br, :].rearrange("p (r c) -> p r c", c=PW)[
                        :, 8 * t + PADR:8 * t + 8 + PADR, PADC:PADC + W
                    ]
                    src = ps.rearrange("p (r c) -> p r c", c=W)
                if (m + t) % 2 == 0:
                    nc.scalar.activation(
                        out=dest, in_=src,
                        func=mybir.ActivationFunctionType.Identity,
                        bias=bq_sb[:, m:m + 1], scale=1.0,
                    )
                else:
                    nc.vector.tensor_scalar_add(dest, src, bq_sb[:, m:m + 1])

        # ---- phases 2+3, branch-sequential, 2-block-paired units ----
        UNITS = [(0, 2), (2, 2), (4, 2), (6, 1)]   # (t0, n_blocks)
        for br in range(3):
            d = DILS[br]
            Kg = K_sb[:, br, :].rearrange("p (r c) -> p r c", c=PW)
            Vg = V_sb[:, br, :].rearrange("p (r c) -> p r c", c=PW)
            Pc = pcp.tile([18, NPIX], BF16, tag="pc")
            den = denp.tile([14, BLK], F32, tag="den")
            rec = recp.tile([14, BLK], BF16, tag="rec")

            # -- 2A: scores + exp + denominators --
            for t0, nb in UNITS:
                L = nb * BLK
                q_in = Q_sb[:, br, t0 * BLK:t0 * BLK + L].rearrange(
                    "p (r c) -> p r c", c=W)
                sps = [ppb.tile([18, BLK], F32, tag="s18", name=f"sp{ti}")
                       for ti in range(nb)]
                for j, (a, b) in enumerate(SHIFTS):
                    prod = prodp.tile([96, 16, W], BF16, tag="prod")
                    k_in = Kg[:, 8 * t0 + PADR + a * d:
                              8 * t0 + 8 * nb + PADR + a * d,
                              PADC + b * d: PADC + b * d + W]
                    nc.vector.tensor_tensor(prod[:, 0:8 * nb, :], q_in, k_in,
                                            op=AluOpType.mult)
                    pf = prod.rearrange("p r c -> p (r c)")
                    for ti in range(nb):
                        nc.tensor.matmul(
                            sps[ti],
                            lhsT=ones18_sb[:, j, :],
                            rhs=pf[:, ti * BLK:(ti + 1) * BLK],
                            start=(j == 0), stop=(j == 8),
                        )
                for ti in range(nb):
                    t = t0 + ti
                    sl = slice(t * BLK, (t + 1) * BLK)
                    nc.scalar.activation(
                        out=Pc[:, sl], in_=sps[ti],
                        func=mybir.ActivationFunctionType.Exp, scale=SCALE,
                    )
                    dp = ppb.tile([18, BLK], F32, tag="s18")
                    nc.tensor.matmul(dp, lhsT=selden_sb, rhs=Pc[:, sl],
                                     start=True, stop=True)
                    # engines can't write at partition base 2t: stage + DMA
                    dst2 = dstp.tile([2, BLK], F32, tag="dst2")
                    nc.scalar.activation(
                        out=dst2, in_=dp[0:2, :],
                        func=mybir.ActivationFunctionType.Copy,
                    )
                    nc.scalar.dma_start(out=den[2 * t:2 * t + 2, :], in_=dst2)
            # batched per-branch reciprocal (14 rows x 448)
            nc.vector.reciprocal(den, den)
            nc.vector.tensor_copy(rec, den)

            # -- 2B/3: broadcast, AV, add tree, normalize --
            for t0, nb in UNITS:
                L = nb * BLK
                sl = slice(t0 * BLK, t0 * BLK + L)
                rec96 = rpp.tile([96, 2, BLK], BF16, tag="rec96")
                for ti in range(nb):
                    for h in range(2):
                        _bcast_dma(nc.sync, rec96[48 * h:48 * h + 48, ti, :],
                                   rec[2 * (t0 + ti) + h:
                                       2 * (t0 + ti) + h + 1, :], 48, BLK)
                tmp = tmpp.tile([96, 9, 2 * BLK], BF16, tag="tmp")
                for j in range(9):
                    for h in range(2):
                        eng = nc.sync if (j + h) % 2 == 0 else nc.scalar
                        _bcast_dma(eng, tmp[48 * h:48 * h + 48, j, 0:L],
                                   Pc[2 * j + h:2 * j + h + 1, sl], 48, L)
                # AV products, one op per kernel-row group (3 taps each)
                for a in range(3):
                    v_in = Vg[:, 8 * t0 + PADR + (a - 1) * d:
                              8 * t0 + 8 * nb + PADR + (a - 1) * d,
                              PADC - d:PADC - d + W]
                    v_ap = bass.AP(
                        tensor=v_in.tensor, offset=v_in.offset,
                        ap=[list(v_in.ap[0]),
                            [d, 3], [PW, 8 * nb], [1, W]],
                    )
                    tslc = tmp[:, 3 * a:3 * a + 3, 0:L].rearrange(
                        "p t (r c) -> p t r c", c=W)
                    nc.vector.tensor_tensor(tslc, tslc, v_ap, op=AluOpType.mult)
                # in-place add tree: 9 taps -> 1 (GpSimd takes one level-1 op)
                nc.gpsimd.tensor_tensor(tmp[:, 0:2, 0:L], tmp[:, 0:2, 0:L],
                                        tmp[:, 2:4, 0:L], op=AluOpType.add)
                nc.vector.tensor_tensor(tmp[:, 4:6, 0:L], tmp[:, 4:6, 0:L],
                                        tmp[:, 6:8, 0:L], op=AluOpType.add)
                nc.vector.tensor_tensor(tmp[:, 0:2, 0:L], tmp[:, 0:2, 0:L],
                                        tmp[:, 4:6, 0:L], op=AluOpType.add)
                nc.vector.tensor_tensor(tmp[:, 0:1, 0:L], tmp[:, 0:1, 0:L],
                                        tmp[:, 1:2, 0:L], op=AluOpType.add)
                with nc.allow_low_precision(reason="9-tap bf16 tree sum"):
                    nc.vector.tensor_tensor(tmp[:, 1:2, 0:L], tmp[:, 0:1, 0:L],
                                            tmp[:, 8:9, 0:L], op=AluOpType.add)
                # normalize by the softmax denominator at the very end
                nc.vector.tensor_tensor(
                    yb_sb[:, br, sl], tmp[:, 1, 0:L],
                    rec96[:, 0:nb, :].rearrange("p a c -> p (a c)"),
                    op=AluOpType.mult)

        # ---- phase 4: output projection + write out ----
        for t in range(25):
            size = min(128, NPIX - t * 128)
            py_full = bigp.tile([128, BLK], F32, tag="big")
            py = py_full[:, 0:C]
            for br in range(3):
                nc.tensor.matmul(
                    py[:size, :],
                    lhsT=yb_sb[:, br, t * 128:t * 128 + size],
                    rhs=wp_sb[:, br, :],
                    start=(br == 0), stop=False,
                )
            # rank-1 bias add: ones column x b_proj row
            nc.tensor.matmul(
                py[:size, :],
                lhsT=ones1_sb[:, 0:size],
                rhs=bp_sb[:, :],
                start=False, stop=True,
            )
            yo_t = yop.tile([128, C], F32, tag="yo")
            nc.scalar.activation(out=yo_t[:size, :], in_=py[:size, :],
                                 func=mybir.ActivationFunctionType.Copy)
            nc.sync.dma_start(out=y[t * 128:t * 128 + size, :],
                              in_=yo_t[:size, :])

    nc.compile()
    return nc


def host_inputs(x, w_qkv, b_qkv, w_proj, b_proj):
    """Numpy prep of per-core + shared input arrays (keys match dram names)."""
    x = np.asarray(x, np.float32)
    w_qkv = np.asarray(w_qkv, np.float32)
    b_qkv = np.asarray(b_qkv, np.float32)
    w_proj = np.asarray(w_proj, np.float32)
    b_proj = np.asarray(b_proj, np.float32)

    # xT per core: [96, 3, NPIX]
    xT_all = x.reshape(B, NPIX, C).transpose(0, 2, 1)          # [B, C, NPIX]
    xT_all = xT_all.reshape(B, 3, 96, NPIX).transpose(0, 2, 1, 3)  # [B,96,3,NPIX]
    xT_all = np.ascontiguousarray(xT_all).astype(NPBF16)

    # wq: [cin96, m, k, cout96] = w_qkv[m*96+cout, k*96+cin]
    w3 = w_qkv.reshape(9, 96, 3, 96)                            # [m,cout,k,cin]
    wq_h = np.ascontiguousarray(w3.transpose(3, 0, 2, 1)).astype(NPBF16)
    bq_h = np.ascontiguousarray(b_qkv.reshape(9, 96).T).astype(np.float32)

    # wp: [cin96, branch, cout] = w_proj[cout, branch*96+cin]
    wp_h = np.ascontiguousarray(
        w_proj.reshape(C, 3, 96).transpose(2, 1, 0)).astype(NPBF16)
    bp_h = np.ascontiguousarray(b_proj[None, :]).astype(NPBF16)

    ones18_h = np.zeros((96, 9, 18), NPBF16)
    for j in range(9):
        ones18_h[0:48, j, 2 * j] = 1
        ones18_h[48:96, j, 2 * j + 1] = 1

    selden_h = np.zeros((18, 18), NPBF16)
    for j in range(9):
        for h in range(2):
            for j2 in range(9):
                selden_h[2 * j + h, 2 * j2 + h] = 1

    ones1_h = np.ones((1, 128), NPBF16)

    shared = dict(wq=wq_h, bq=bq_h, wp=wp_h, bp=bp_h, ones18=ones18_h,
                  selden=selden_h, ones1=ones1_h)
    in_maps = [dict(shared, xT=np.ascontiguousarray(xT_all[i]))
               for i in range(N_CORES)]
    return in_maps


_NC_CACHE = {}


def kernel(x, w_qkv, b_qkv, w_proj, b_proj):
    if "nc" not in _NC_CACHE:
        _NC_CACHE["nc"] = build_nc()
    nc = _NC_CACHE["nc"]
    in_maps = host_inputs(x, w_qkv, b_qkv, w_proj, b_proj)
    res = run_bass_kernel_spmd(nc, in_maps, list(range(N_CORES)))
    out = np.stack([res.results[i]["y"] for i in range(N_CORES)], axis=0)
    return out.reshape(B, H, W, C).astype(np.float32)


if __name__ == "__main__":
    print("built nc ok" if build_nc() else "")
